# revision 53
# baseline (speedup 1.0000x reference)
"""DiagonalBiLSTM Trainium2 kernel (v4).

Full inputs in, full output out. 8-way data-parallel over the 1024 flattened
(batch, height) scan rows. Cross-core h_next coupling handled with redundant
ghost rows that SHRINK with the step index (ghost j only influences own rows
for steps d <= 125 - j, so at step d only nd = 128 + max(0, 126-d) columns
are live).

The two scan directions are processed as alternating phases: they are
independent recurrences, so while one direction runs its pointwise tail
(activations + c/h update), the Tensor engine executes the other
direction's matmuls — hiding the serial chain and keeping the PE p-state
ramped.

The masked 1x1 input conv (wm @ x + b_i2s + b_s2s) is precomputed on the
host and streamed per step. The gate bias (b_ih + b_hh) is injected into
the gate PSUM by a "bias matmul" (lhsT rows 0/1 hold the two tile biases,
rhs is a 0/1 selector) which doubles as the PSUM-group opener and as
filler work that keeps the PE warm during the other phase's tail. With the
bias in PSUM, activations run bias-free over PAIRS of gate tiles (4 ops
per phase instead of 8).

All matmul/elementwise tensors are bf16 (PSUM fp32); host-sim validated
rel err ~8e-3 vs the 2e-2 gate.

Gate tile layout [i0,i1,f0,f1,o0,o1,g0,g1] (pairs i,f,o,g), emitted g
first so the c/h chain starts as early as possible, o last.
"""

import numpy as np
import ml_dtypes

BF16 = ml_dtypes.bfloat16

B, CIN, COUT, H, W, DC = 16, 256, 256, 64, 64, 3
WD = 2 * W - 1          # 127 diagonal steps
NCORES = 8
OWN = 128               # own rows per core (2 batches)
GHOST = 126             # max redundant ghost rows
NCOL = 256              # padded col count per dir
NKC = 2                 # channel chunks (256 = 2*128)
NGT = 8                 # gate m-tiles (1024 = 8*128)

_COMPILED = {}


def _nd(d):
    return OWN + max(0, GHOST - d)


# ----------------------------------------------------------------- host prep

def _i2s_mask_np():
    oc = np.arange(COUT) % DC
    ic = np.arange(CIN) % DC
    return (ic[None, :] <= oc[:, None]).astype(np.float32)


def _wT_tiles(w, nmt):
    # [out=nmt*128, in=256] -> lhsT tile array [k=128, kc=2, mt=nmt, m=128]
    return np.ascontiguousarray(
        w.T.reshape(NKC, 128, nmt, 128).transpose(1, 0, 2, 3)).astype(BF16)


def _diag_pack(y_loc, bias_u):
    """y_loc [4, 256, 64, 64] (input-conv output for local batches, already
    W-flipped for the R dir) -> [WD, NKC, 128, 256cols], col j = local row
    (b_loc*64 + h), value y[b, c, h, d - h] + bias_u (bias everywhere)."""
    xs = np.zeros((WD, 4, CIN, H), np.float32)
    for h in range(H):
        xs[h:h + W, :, :, h] = y_loc[:, :, h, :].transpose(2, 0, 1)
    xs = xs.transpose(0, 2, 1, 3).reshape(WD, CIN, 4 * H)
    xs += bias_u[None, :, None]
    return np.ascontiguousarray(xs.reshape(WD, NKC, 128, 4 * H))


def _prep_inputs(x, w_i2s, b_i2s, w_ih, b_ih, b_hh, k0, k1, b_s2s):
    wm = (w_i2s * _i2s_mask_np()).astype(np.float32)
    k0_t = _wT_tiles(k0.astype(np.float32), 2)
    k1_t = _wT_tiles(k1.astype(np.float32), 2)

    # gate reorder: original rows [i, f, g, o] blocks of 256 -> [i, f, o, g]
    def perm_rows(v):
        return np.concatenate([v[0:512], v[768:1024], v[512:768]], axis=0)

    wih_t = _wT_tiles(perm_rows(w_ih.astype(np.float32)), 8)
    bias_u = (b_i2s + b_s2s).astype(np.float32)
    bias_g = perm_rows((b_ih + b_hh).astype(np.float32))

    # bias matmul lhsT: per pair p, rows 0/1 hold the biases of tiles
    # 2p / 2p+1, all other rows zero.  [k=128, pair=4, m=128]
    bias_w = np.zeros((128, 4, 128), np.float32)
    for p in range(4):
        bias_w[0, p, :] = bias_g[(2 * p) * 128:(2 * p + 1) * 128]
        bias_w[1, p, :] = bias_g[(2 * p + 1) * 128:(2 * p + 2) * 128]
    # selector rhs: sel[k, tt, col] = 1 iff k == tt
    sel = np.zeros((128, 2, NCOL), np.float32)
    sel[0, 0, :] = 1.0
    sel[1, 1, :] = 1.0

    misc_all = np.zeros((NCORES, 128, 4), np.float32)
    misc_all[:, :, 0] = 1.0
    misc_all[7, :, 0] = 0.0                           # core 7: zero ghost0 h

    # host input-conv (masked 1x1), bias folded in at pack time
    y = np.einsum('oc,bchw->bohw', wm, x.astype(np.float32), optimize=True)
    yf = y[:, :, :, ::-1]                             # W-flip for R direction
    in_maps = []
    for c in range(NCORES):
        yloc = np.zeros((4, CIN, H, W), np.float32)
        yfloc = np.zeros((4, CIN, H, W), np.float32)
        nb = min(4, B - 2 * c)
        yloc[:nb] = y[2 * c:2 * c + nb]
        yfloc[:nb] = yf[2 * c:2 * c + nb]
        dl = _diag_pack(yloc, bias_u)                 # [WD, 2, 128, 256]
        dr = _diag_pack(yfloc, bias_u)
        xd = np.stack([dl, dr], axis=3)               # [WD, 2, 128, 2, 256]
        in_maps.append({
            "xd": np.ascontiguousarray(xd.astype(BF16)),
            "k0t": k0_t, "k1t": k1_t, "wih": wih_t,
            "biasw": bias_w.astype(BF16),
            "sel": sel.astype(BF16),
            "misc": misc_all[c],
        })
    return in_maps


# ------------------------------------------------------- reference-free host
# numpy replica of the device program (bf16 rounding emulated)

def _core_sim(im, nsteps=WD):
    def r(v):
        return np.asarray(v).astype(BF16).astype(np.float32)

    xd = np.asarray(im["xd"]).astype(np.float32)   # already bf16-rounded
    misc = im["misc"]

    def unT(t, nmt):
        return t.astype(np.float32).transpose(1, 0, 2, 3).reshape(
            CIN, nmt * 128).T

    k0, k1, wih = (unT(im["k0t"], 2), unT(im["k1t"], 2), unT(im["wih"], 8))
    bw = im["biasw"].astype(np.float32)
    bias_g = np.concatenate(
        [np.stack([bw[0, p], bw[1, p]]).reshape(-1) for p in range(4)])
    s = misc[0, 0]

    def sig(v):
        return 1.0 / (1.0 + np.exp(-v))

    h = np.zeros((CIN, 2, NCOL), np.float32)
    cst = np.zeros((CIN, 2, NCOL), np.float32)
    out = np.zeros((WD, 2, 128, 2, OWN), np.float32)
    for d in range(nsteps):
        nd = _nd(d)
        xs = xd[d].reshape(CIN, 2, NCOL)[:, :, 0:nd]
        u = r(xs + (k0 @ h[:, :, 0:nd].reshape(CIN, -1)).reshape(CIN, 2, nd)
              + (k1 @ h[:, :, 1:nd + 1].reshape(CIN, -1)).reshape(CIN, 2, nd))
        g = (wih @ u.reshape(CIN, -1)).reshape(4 * CIN, 2, nd) \
            + bias_g[:, None, None]
        gi, gf = r(sig(g[0:256])), r(sig(g[256:512]))
        go, gg = r(sig(g[512:768])), r(np.tanh(g[768:1024]))
        if nd > OWN:
            go[:, :, OWN] *= s          # ghost-boundary zero via sig(o)
        t1 = r(gi * gg)
        cst[:, :, 0:nd] = r(r(gf * cst[:, :, 0:nd]) + t1)
        h[:, :, 0:nd] = r(go * r(np.tanh(cst[:, :, 0:nd])))
        out[d] = h.reshape(2, 128, 2, NCOL)[:, :, :, 0:OWN]
    return out


# ----------------------------------------------------------- output assembly

def _assemble(core_outs):
    # core_outs: list of [WD, 2, 128, 2, OWN] -> hs [2dir, WD, 256ch, 1024]
    hs = np.zeros((2, WD, CIN, B * H), np.float32)
    for c, o in enumerate(core_outs):
        o = np.asarray(o).astype(np.float32)
        hs[:, :, :, c * OWN:(c + 1) * OWN] = (
            o.transpose(3, 0, 1, 2, 4).reshape(2, WD, CIN, OWN))

    def unscramble(hd):             # [WD, 256ch, 1024rows] -> [B, COUT, H, WD]
        a = hd.transpose(0, 2, 1).reshape(WD, B, COUT, H)
        return a.transpose(1, 2, 3, 0)

    def unshift(a):                 # [B, COUT, H, WD] -> [B, COUT, H, W]
        rows = np.arange(H)[:, None]
        cols = rows + np.arange(W)[None, :]
        return a[:, :, rows, cols]

    left = unshift(unscramble(hs[0]))
    right = unshift(unscramble(hs[1]))[:, :, :, ::-1]
    right = np.concatenate(
        [np.zeros_like(right[:, :, :1, :]), right[:, :, :-1, :]], axis=2)
    return left + right


# --------------------------------------------------------------- bass kernel

def _build(nsteps=WD):
    import concourse.bacc as bacc
    import concourse.mybir as mybir
    import concourse.tile as tile
    from concourse._compat import get_trn_type

    f32 = mybir.dt.float32
    bf = mybir.dt.bfloat16
    AF = mybir.ActivationFunctionType

    nc = bacc.Bacc(get_trn_type() or "TRN2", target_bir_lowering=False,
                   debug=False)
    xd = nc.dram_tensor("xd", [WD, NKC, 128, 2, NCOL], bf,
                        kind="ExternalInput")
    k0t = nc.dram_tensor("k0t", [128, NKC, 2, 128], bf, kind="ExternalInput")
    k1t = nc.dram_tensor("k1t", [128, NKC, 2, 128], bf, kind="ExternalInput")
    wih = nc.dram_tensor("wih", [128, NKC, NGT, 128], bf,
                         kind="ExternalInput")
    biasw = nc.dram_tensor("biasw", [128, 4, 128], bf, kind="ExternalInput")
    seld = nc.dram_tensor("sel", [128, 2, NCOL], bf, kind="ExternalInput")
    misc = nc.dram_tensor("misc", [128, 4], f32, kind="ExternalInput")
    hs_out = nc.dram_tensor("hs", [WD, NKC, 128, 2, OWN], bf,
                            kind="ExternalOutput")

    # gate tile layout [i0,i1,f0,f1,o0,o1,g0,g1]; act pairs p: 0=i 1=f 2=o 3=g
    # pair emission order g, f, i, o: cmul only needs f (2nd act slot), so
    # the DVE chain starts earlier and the next phase's u-moves slot in
    # before the chain tail.
    PEMIT = [3, 0, 1, 2]

    with tile.TileContext(nc) as tc:
        with (
            tc.tile_pool(name="wpool", bufs=1) as wpool,
            tc.tile_pool(name="state", bufs=1) as state,
            tc.tile_pool(name="xpool", bufs=6) as xpool,
            tc.tile_pool(name="upool", bufs=4) as upool,
            tc.tile_pool(name="apool", bufs=4) as apool,
            tc.tile_pool(name="tpool", bufs=4) as tpool,
            tc.tile_pool(name="psum", bufs=6, space="PSUM") as psum,
            tc.tile_pool(name="upsum", bufs=2, space="PSUM") as upsum,
        ):
            k0_t = wpool.tile([128, NKC, 2, 128], bf, tag="k0")
            k1_t = wpool.tile([128, NKC, 2, 128], bf, tag="k1")
            wih_t = wpool.tile([128, NKC, NGT, 128], bf, tag="wih")
            bw_t = wpool.tile([128, 4, 128], bf, tag="bw")
            sel_t = wpool.tile([128, 2, NCOL], bf, tag="sel")
            misc_t = wpool.tile([128, 4], f32, tag="misc")
            nc.sync.dma_start(k0_t[:], k0t[:])
            nc.sync.dma_start(k1_t[:], k1t[:])
            nc.sync.dma_start(wih_t[:], wih[:])
            nc.sync.dma_start(bw_t[:], biasw[:])
            nc.sync.dma_start(sel_t[:], seld[:])
            nc.sync.dma_start(misc_t[:], misc[:])

            h = state.tile([128, NKC, 2, NCOL], bf, tag="h")
            cs = state.tile([128, NKC, 2, NCOL], bf, tag="c")
            nc.any.memset(h[:], 0.0)
            nc.any.memset(cs[:], 0.0)

            def emit_xs(d, di):
                nd = _nd(d)
                xs = xpool.tile([128, NKC, NCOL], bf, tag="xs")
                nc.sync.dma_start(xs[:, :, 0:nd],
                                  xd[d][:, :, di, 0:nd].transpose((1, 0, 2)))
                return xs

            def emit_head(d, di):
                """Open phase (d, di): psum allocs + bias matmuls + kmms +
                u-moves. Returns state for emit_gates."""
                nd = _nd(d)
                up0 = upsum.tile([128, NCOL], f32, tag="up")
                up1 = upsum.tile([128, NCOL], f32, tag="up")
                ups = [up0, up1]
                xs = emit_xs(d, di)
                u = upool.tile([128, NKC, NCOL], bf, tag="u")
                # separate up tiles per m-half: tile-granular dependency
                # tracking would otherwise serialize m1's matmuls behind
                # m0's u-move read. Per-half u-moves let the kc0 gate
                # matmuls start after only half the move.
                for m in range(NKC):
                    for kc in range(NKC):
                        nc.tensor.matmul(
                            ups[m][:, 0:nd], k0_t[:, kc, m, :],
                            h[:, kc, di, 0:nd],
                            start=(kc == 0), stop=False)
                    for kc in range(NKC):
                        nc.tensor.matmul(
                            ups[m][:, 0:nd], k1_t[:, kc, m, :],
                            h[:, kc, di, 1:nd + 1],
                            start=False, stop=(kc == NKC - 1))
                    nc.vector.tensor_add(
                        u[:, m, 0:nd], ups[m][:, 0:nd], xs[:, m, 0:nd])
                pairs = {}
                for p in PEMIT:
                    gp = psum.tile([128, 2, NCOL], f32, tag="ps")
                    nc.tensor.matmul(gp[:, :, 0:nd], bw_t[:, p, :],
                                     sel_t[:, :, 0:nd],
                                     start=True, stop=False)
                    pairs[p] = gp
                return (d, di, pairs, u)

            def emit_pair(head, p, acts):
                d, di, pairs, u = head
                nd = _nd(d)
                gp = pairs[p]
                import contextlib
                prio = (tc.high_priority() if p == 3
                        else contextlib.nullcontext())
                with prio:
                    for tt in range(2):
                        t = 2 * p + tt
                        for kc in range(NKC):
                            nc.tensor.matmul(
                                gp[:, tt, 0:nd], wih_t[:, kc, t, :],
                                u[:, kc, 0:nd],
                                start=False, stop=(kc == NKC - 1))
                fn = AF.Tanh if p == 3 else AF.Sigmoid
                nc.scalar.activation(acts[:, 2 * p:2 * p + 2, 0:nd],
                                     gp[:, :, 0:nd], fn)
                if p == 2:
                    # zero the o-gate's ghost-boundary column (core 7 only,
                    # scale=0) BEFORE hmul, so h is final at hmul end and
                    # the old post-hmul fixup leaves the critical path.
                    nc.vector.tensor_scalar_mul(
                        acts[:, 4:6, OWN:OWN + 1], acts[:, 4:6, OWN:OWN + 1],
                        misc_t[:, 0:1])

            def emit_chain(d, di, acts):
                """Pointwise c/h chain for phase (d, di), m-split so the
                c-tanh is ready as soon as the Act engine frees."""
                nd = _nd(d)
                t1 = tpool.tile([128, 2, NCOL], bf, tag="t1")
                ct = tpool.tile([128, 2, NCOL], bf, tag="ct")
                nc.vector.tensor_mul(t1[:, :, 0:nd],
                                     acts[:, 0:2, 0:nd],
                                     acts[:, 6:8, 0:nd])
                nc.vector.tensor_mul(cs[:, :, di, 0:nd],
                                     cs[:, :, di, 0:nd],
                                     acts[:, 2:4, 0:nd])
                nc.vector.tensor_add(cs[:, :, di, 0:nd],
                                     cs[:, :, di, 0:nd],
                                     t1[:, :, 0:nd])
                nc.scalar.activation(ct[:, :, 0:nd],
                                     cs[:, :, di, 0:nd], AF.Tanh)
                nc.vector.tensor_mul(h[:, :, di, 0:nd],
                                     acts[:, 4:6, 0:nd],
                                     ct[:, :, 0:nd])
                nc.sync.dma_start(
                    hs_out[d][:, :, di].transpose((1, 0, 2)),
                    h[:, :, di, 0:OWN])

            # software-pipelined emission: the next phase's kmms/u-moves are
            # emitted between this phase's i and o gate pairs, so the PE's
            # priority tie-break runs them ahead of the o-gate matmuls and
            # the next phase's gate work can start a full act-slot earlier.
            phases = [(d, di) for d in range(nsteps) for di in range(2)]
            head = emit_head(*phases[0])
            for idx, (d, di) in enumerate(phases):
                acts = apool.tile([128, NGT, NCOL], bf, tag="acts")
                for p in PEMIT[:3]:                       # g, f, i
                    emit_pair(head, p, acts)
                nxt = (emit_head(*phases[idx + 1])
                       if idx + 1 < len(phases) else None)
                emit_pair(head, PEMIT[3], acts)           # o
                emit_chain(d, di, acts)
                head = nxt

    nc.finalize()
    return nc


def _get_compiled(nsteps=WD):
    if nsteps not in _COMPILED:
        _COMPILED[nsteps] = _build(nsteps)
    return _COMPILED[nsteps]


# ------------------------------------------------------------------- driver

def kernel(x, w_i2s, b_i2s, w_ih, b_ih, b_hh, k0, k1, b_s2s):
    from concourse.bass_utils import run_bass_kernel_spmd

    in_maps = _prep_inputs(np.asarray(x, np.float32), np.asarray(w_i2s),
                           np.asarray(b_i2s), np.asarray(w_ih),
                           np.asarray(b_ih), np.asarray(b_hh),
                           np.asarray(k0), np.asarray(k1), np.asarray(b_s2s))
    nc = _get_compiled()
    res = run_bass_kernel_spmd(nc, in_maps, list(range(NCORES)))
    return _assemble([res.results[c]["hs"] for c in range(NCORES)])


def kernel_numpy(x, w_i2s, b_i2s, w_ih, b_ih, b_hh, k0, k1, b_s2s):
    """Host-only replica of the device program (debug path)."""
    in_maps = _prep_inputs(np.asarray(x, np.float32), np.asarray(w_i2s),
                           np.asarray(b_i2s), np.asarray(w_ih),
                           np.asarray(b_ih), np.asarray(b_hh),
                           np.asarray(k0), np.asarray(k1), np.asarray(b_s2s))
    return _assemble([_core_sim(im) for im in in_maps])


# revision 54
# speedup vs baseline: 1.0237x; 1.0237x over previous
"""DiagonalBiLSTM Trainium2 kernel (v4).

Full inputs in, full output out. 8-way data-parallel over the 1024 flattened
(batch, height) scan rows. Cross-core h_next coupling handled with redundant
ghost rows that SHRINK with the step index (ghost j only influences own rows
for steps d <= 125 - j, so at step d only nd = 128 + max(0, 126-d) columns
are live).

The two scan directions are processed as alternating phases: they are
independent recurrences, so while one direction runs its pointwise tail
(activations + c/h update), the Tensor engine executes the other
direction's matmuls — hiding the serial chain and keeping the PE p-state
ramped.

The masked 1x1 input conv (wm @ x + b_i2s + b_s2s) is precomputed on the
host and streamed per step. The gate bias (b_ih + b_hh) is injected into
the gate PSUM by a "bias matmul" (lhsT rows 0/1 hold the two tile biases,
rhs is a 0/1 selector) which doubles as the PSUM-group opener and as
filler work that keeps the PE warm during the other phase's tail. With the
bias in PSUM, activations run bias-free over PAIRS of gate tiles (4 ops
per phase instead of 8).

All matmul/elementwise tensors are bf16 (PSUM fp32); host-sim validated
rel err ~8e-3 vs the 2e-2 gate.

Gate tile layout [i0,i1,f0,f1,o0,o1,g0,g1] (pairs i,f,o,g), emitted g
first so the c/h chain starts as early as possible, o last.
"""

import numpy as np
import ml_dtypes

BF16 = ml_dtypes.bfloat16

B, CIN, COUT, H, W, DC = 16, 256, 256, 64, 64, 3
WD = 2 * W - 1          # 127 diagonal steps
NCORES = 8
OWN = 128               # own rows per core (2 batches)
GHOST = 126             # max redundant ghost rows
NCOL = 256              # padded col count per dir
NKC = 2                 # channel chunks (256 = 2*128)
NGT = 8                 # gate m-tiles (1024 = 8*128)

_COMPILED = {}


def _nd(d):
    return OWN + max(0, GHOST - d)


# ----------------------------------------------------------------- host prep

def _i2s_mask_np():
    oc = np.arange(COUT) % DC
    ic = np.arange(CIN) % DC
    return (ic[None, :] <= oc[:, None]).astype(np.float32)


def _wT_tiles(w, nmt):
    # [out=nmt*128, in=256] -> lhsT tile array [k=128, kc=2, mt=nmt, m=128]
    return np.ascontiguousarray(
        w.T.reshape(NKC, 128, nmt, 128).transpose(1, 0, 2, 3)).astype(BF16)


def _diag_pack(y_loc, bias_u):
    """y_loc [4, 256, 64, 64] (input-conv output for local batches, already
    W-flipped for the R dir) -> [WD, NKC, 128, 256cols], col j = local row
    (b_loc*64 + h), value y[b, c, h, d - h] + bias_u (bias everywhere)."""
    xs = np.zeros((WD, 4, CIN, H), np.float32)
    for h in range(H):
        xs[h:h + W, :, :, h] = y_loc[:, :, h, :].transpose(2, 0, 1)
    xs = xs.transpose(0, 2, 1, 3).reshape(WD, CIN, 4 * H)
    xs += bias_u[None, :, None]
    return np.ascontiguousarray(xs.reshape(WD, NKC, 128, 4 * H))


def _prep_inputs(x, w_i2s, b_i2s, w_ih, b_ih, b_hh, k0, k1, b_s2s):
    wm = (w_i2s * _i2s_mask_np()).astype(np.float32)
    k0_t = _wT_tiles(k0.astype(np.float32), 2)
    k1_t = _wT_tiles(k1.astype(np.float32), 2)

    # gate reorder: original rows [i, f, g, o] blocks of 256 -> [i, f, o, g]
    def perm_rows(v):
        return np.concatenate([v[0:512], v[768:1024], v[512:768]], axis=0)

    wih_t = _wT_tiles(perm_rows(w_ih.astype(np.float32)), 8)
    bias_u = (b_i2s + b_s2s).astype(np.float32)
    bias_g = perm_rows((b_ih + b_hh).astype(np.float32))

    # bias matmul lhsT: per pair p, rows 0/1 hold the biases of tiles
    # 2p / 2p+1, all other rows zero.  [k=128, pair=4, m=128]
    bias_w = np.zeros((128, 4, 128), np.float32)
    for p in range(4):
        bias_w[0, p, :] = bias_g[(2 * p) * 128:(2 * p + 1) * 128]
        bias_w[1, p, :] = bias_g[(2 * p + 1) * 128:(2 * p + 2) * 128]
    # selector rhs: sel[k, tt, col] = 1 iff k == tt
    sel = np.zeros((128, 2, NCOL), np.float32)
    sel[0, 0, :] = 1.0
    sel[1, 1, :] = 1.0

    misc_all = np.zeros((NCORES, 128, 4), np.float32)
    misc_all[:, :, 0] = 1.0
    misc_all[7, :, 0] = 0.0                           # core 7: zero ghost0 h

    # host input-conv (masked 1x1), bias folded in at pack time
    y = np.einsum('oc,bchw->bohw', wm, x.astype(np.float32), optimize=True)
    yf = y[:, :, :, ::-1]                             # W-flip for R direction
    in_maps = []
    for c in range(NCORES):
        yloc = np.zeros((4, CIN, H, W), np.float32)
        yfloc = np.zeros((4, CIN, H, W), np.float32)
        nb = min(4, B - 2 * c)
        yloc[:nb] = y[2 * c:2 * c + nb]
        yfloc[:nb] = yf[2 * c:2 * c + nb]
        dl = _diag_pack(yloc, bias_u)                 # [WD, 2, 128, 256]
        dr = _diag_pack(yfloc, bias_u)
        xd = np.stack([dl, dr], axis=3)               # [WD, 2, 128, 2, 256]
        in_maps.append({
            "xd": np.ascontiguousarray(xd.astype(BF16)),
            "k0t": k0_t, "k1t": k1_t, "wih": wih_t,
            "biasw": bias_w.astype(BF16),
            "sel": sel.astype(BF16),
            "misc": misc_all[c],
        })
    return in_maps


# ------------------------------------------------------- reference-free host
# numpy replica of the device program (bf16 rounding emulated)

def _core_sim(im, nsteps=WD):
    def r(v):
        return np.asarray(v).astype(BF16).astype(np.float32)

    xd = np.asarray(im["xd"]).astype(np.float32)   # already bf16-rounded
    misc = im["misc"]

    def unT(t, nmt):
        return t.astype(np.float32).transpose(1, 0, 2, 3).reshape(
            CIN, nmt * 128).T

    k0, k1, wih = (unT(im["k0t"], 2), unT(im["k1t"], 2), unT(im["wih"], 8))
    bw = im["biasw"].astype(np.float32)
    bias_g = np.concatenate(
        [np.stack([bw[0, p], bw[1, p]]).reshape(-1) for p in range(4)])
    s = misc[0, 0]

    def sig(v):
        return 1.0 / (1.0 + np.exp(-v))

    h = np.zeros((CIN, 2, NCOL), np.float32)
    cst = np.zeros((CIN, 2, NCOL), np.float32)
    out = np.zeros((WD, 2, 128, 2, OWN), np.float32)
    for d in range(nsteps):
        nd = _nd(d)
        xs = xd[d].reshape(CIN, 2, NCOL)[:, :, 0:nd]
        u = r(xs + (k0 @ h[:, :, 0:nd].reshape(CIN, -1)).reshape(CIN, 2, nd)
              + (k1 @ h[:, :, 1:nd + 1].reshape(CIN, -1)).reshape(CIN, 2, nd))
        g = (wih @ u.reshape(CIN, -1)).reshape(4 * CIN, 2, nd) \
            + bias_g[:, None, None]
        gi, gf = r(sig(g[0:256])), r(sig(g[256:512]))
        go, gg = r(sig(g[512:768])), r(np.tanh(g[768:1024]))
        if nd > OWN:
            go[:, :, OWN] *= s          # ghost-boundary zero via sig(o)
        t1 = r(gi * gg)
        cst[:, :, 0:nd] = r(r(gf * cst[:, :, 0:nd]) + t1)
        h[:, :, 0:nd] = r(go * r(np.tanh(cst[:, :, 0:nd])))
        out[d] = h.reshape(2, 128, 2, NCOL)[:, :, :, 0:OWN]
    return out


# ----------------------------------------------------------- output assembly

def _assemble(core_outs):
    # core_outs: list of [WD, 2, 128, 2, OWN] -> hs [2dir, WD, 256ch, 1024]
    hs = np.zeros((2, WD, CIN, B * H), np.float32)
    for c, o in enumerate(core_outs):
        o = np.asarray(o).astype(np.float32)
        hs[:, :, :, c * OWN:(c + 1) * OWN] = (
            o.transpose(3, 0, 1, 2, 4).reshape(2, WD, CIN, OWN))

    def unscramble(hd):             # [WD, 256ch, 1024rows] -> [B, COUT, H, WD]
        a = hd.transpose(0, 2, 1).reshape(WD, B, COUT, H)
        return a.transpose(1, 2, 3, 0)

    def unshift(a):                 # [B, COUT, H, WD] -> [B, COUT, H, W]
        rows = np.arange(H)[:, None]
        cols = rows + np.arange(W)[None, :]
        return a[:, :, rows, cols]

    left = unshift(unscramble(hs[0]))
    right = unshift(unscramble(hs[1]))[:, :, :, ::-1]
    right = np.concatenate(
        [np.zeros_like(right[:, :, :1, :]), right[:, :, :-1, :]], axis=2)
    return left + right


# --------------------------------------------------------------- bass kernel

def _build(nsteps=WD):
    import concourse.bacc as bacc
    import concourse.mybir as mybir
    import concourse.tile as tile
    from concourse._compat import get_trn_type

    f32 = mybir.dt.float32
    bf = mybir.dt.bfloat16
    AF = mybir.ActivationFunctionType

    nc = bacc.Bacc(get_trn_type() or "TRN2", target_bir_lowering=False,
                   debug=False)
    xd = nc.dram_tensor("xd", [WD, NKC, 128, 2, NCOL], bf,
                        kind="ExternalInput")
    k0t = nc.dram_tensor("k0t", [128, NKC, 2, 128], bf, kind="ExternalInput")
    k1t = nc.dram_tensor("k1t", [128, NKC, 2, 128], bf, kind="ExternalInput")
    wih = nc.dram_tensor("wih", [128, NKC, NGT, 128], bf,
                         kind="ExternalInput")
    biasw = nc.dram_tensor("biasw", [128, 4, 128], bf, kind="ExternalInput")
    seld = nc.dram_tensor("sel", [128, 2, NCOL], bf, kind="ExternalInput")
    misc = nc.dram_tensor("misc", [128, 4], f32, kind="ExternalInput")
    hs_out = nc.dram_tensor("hs", [WD, NKC, 128, 2, OWN], bf,
                            kind="ExternalOutput")

    # gate tile layout [i0,i1,f0,f1,o0,o1,g0,g1]; act pairs p: 0=i 1=f 2=o 3=g
    # pair emission order g, f, i, o: cmul only needs f (2nd act slot), so
    # the DVE chain starts earlier and the next phase's u-moves slot in
    # before the chain tail.
    PEMIT = [3, 0, 1, 2]

    with tile.TileContext(nc) as tc:
        with (
            tc.tile_pool(name="wpool", bufs=1) as wpool,
            tc.tile_pool(name="state", bufs=1) as state,
            tc.tile_pool(name="xpool", bufs=6) as xpool,
            tc.tile_pool(name="upool", bufs=4) as upool,
            tc.tile_pool(name="apool", bufs=4) as apool,
            tc.tile_pool(name="tpool", bufs=4) as tpool,
            tc.tile_pool(name="psum", bufs=6, space="PSUM") as psum,
            tc.tile_pool(name="upsum", bufs=2, space="PSUM") as upsum,
        ):
            k0_t = wpool.tile([128, NKC, 2, 128], bf, tag="k0")
            k1_t = wpool.tile([128, NKC, 2, 128], bf, tag="k1")
            wih_t = wpool.tile([128, NKC, NGT, 128], bf, tag="wih")
            bw_t = wpool.tile([128, 4, 128], bf, tag="bw")
            sel_t = wpool.tile([128, 2, NCOL], bf, tag="sel")
            misc_t = wpool.tile([128, 4], f32, tag="misc")
            nc.sync.dma_start(k0_t[:], k0t[:])
            nc.sync.dma_start(k1_t[:], k1t[:])
            nc.sync.dma_start(wih_t[:], wih[:])
            nc.sync.dma_start(bw_t[:], biasw[:])
            nc.sync.dma_start(sel_t[:], seld[:])
            nc.sync.dma_start(misc_t[:], misc[:])

            h = state.tile([128, NKC, 2, NCOL], bf, tag="h")
            cs = state.tile([128, NKC, 2, NCOL], bf, tag="c")
            nc.any.memset(h[:], 0.0)
            nc.any.memset(cs[:], 0.0)

            def emit_xs(d, di):
                nd = _nd(d)
                xs = xpool.tile([128, NKC, NCOL], bf, tag="xs")
                nc.sync.dma_start(xs[:, :, 0:nd],
                                  xd[d][:, :, di, 0:nd].transpose((1, 0, 2)))
                return xs

            def emit_head(d, di):
                """Open phase (d, di): psum allocs + bias matmuls + kmms +
                u-moves. Returns state for emit_gates."""
                nd = _nd(d)
                up0 = upsum.tile([128, NCOL], f32, tag="up")
                up1 = upsum.tile([128, NCOL], f32, tag="up")
                ups = [up0, up1]
                xs = emit_xs(d, di)
                u = upool.tile([128, NKC, NCOL], bf, tag="u")
                # separate up tiles per m-half: tile-granular dependency
                # tracking would otherwise serialize m1's matmuls behind
                # m0's u-move read. Per-half u-moves let the kc0 gate
                # matmuls start after only half the move.
                for m in range(NKC):
                    for kc in range(NKC):
                        nc.tensor.matmul(
                            ups[m][:, 0:nd], k0_t[:, kc, m, :],
                            h[:, kc, di, 0:nd],
                            start=(kc == 0), stop=False)
                    for kc in range(NKC):
                        nc.tensor.matmul(
                            ups[m][:, 0:nd], k1_t[:, kc, m, :],
                            h[:, kc, di, 1:nd + 1],
                            start=False, stop=(kc == NKC - 1))
                    nc.vector.tensor_add(
                        u[:, m, 0:nd], ups[m][:, 0:nd], xs[:, m, 0:nd])
                pairs = {}
                for p in PEMIT:
                    gp = psum.tile([128, 2, NCOL], f32, tag="ps")
                    nc.tensor.matmul(gp[:, :, 0:nd], bw_t[:, p, :],
                                     sel_t[:, :, 0:nd],
                                     start=True, stop=False)
                    pairs[p] = gp
                return (d, di, pairs, u)

            def emit_pair(head, p, acts):
                d, di, pairs, u = head
                nd = _nd(d)
                gp = pairs[p]
                import contextlib
                prio = (tc.high_priority() if p == 3
                        else contextlib.nullcontext())
                with prio:
                    for tt in range(2):
                        t = 2 * p + tt
                        for kc in range(NKC):
                            nc.tensor.matmul(
                                gp[:, tt, 0:nd], wih_t[:, kc, t, :],
                                u[:, kc, 0:nd],
                                start=False, stop=(kc == NKC - 1))
                fn = AF.Tanh if p == 3 else AF.Sigmoid
                nc.scalar.activation(acts[:, 2 * p:2 * p + 2, 0:nd],
                                     gp[:, :, 0:nd], fn)
                if p == 2:
                    # zero the o-gate's ghost-boundary column (core 7 only,
                    # scale=0) BEFORE hmul, so h is final at hmul end and
                    # the old post-hmul fixup leaves the critical path.
                    nc.vector.tensor_scalar_mul(
                        acts[:, 4:6, OWN:OWN + 1], acts[:, 4:6, OWN:OWN + 1],
                        misc_t[:, 0:1])

            def emit_chain_pre(d, di, acts):
                """c-update ops: emitted BEFORE the next phase's u-moves so
                they sit ahead in the DVE queue (their act deps resolve
                early — no head-of-line blocking on cadd)."""
                nd = _nd(d)
                t1 = tpool.tile([128, 2, NCOL], bf, tag="t1")
                nc.vector.tensor_mul(t1[:, :, 0:nd],
                                     acts[:, 0:2, 0:nd],
                                     acts[:, 6:8, 0:nd])
                nc.vector.tensor_mul(cs[:, :, di, 0:nd],
                                     cs[:, :, di, 0:nd],
                                     acts[:, 2:4, 0:nd])
                nc.vector.tensor_add(cs[:, :, di, 0:nd],
                                     cs[:, :, di, 0:nd],
                                     t1[:, :, 0:nd])

            def emit_chain(d, di, acts):
                nd = _nd(d)
                ct = tpool.tile([128, 2, NCOL], bf, tag="ct")
                nc.scalar.activation(ct[:, :, 0:nd],
                                     cs[:, :, di, 0:nd], AF.Tanh)
                nc.vector.tensor_mul(h[:, :, di, 0:nd],
                                     acts[:, 4:6, 0:nd],
                                     ct[:, :, 0:nd])
                nc.sync.dma_start(
                    hs_out[d][:, :, di].transpose((1, 0, 2)),
                    h[:, :, di, 0:OWN])

            # software-pipelined emission: the next phase's kmms/u-moves are
            # emitted between this phase's i and o gate pairs, so the PE's
            # priority tie-break runs them ahead of the o-gate matmuls and
            # the next phase's gate work can start a full act-slot earlier.
            phases = [(d, di) for d in range(nsteps) for di in range(2)]
            head = emit_head(*phases[0])
            for idx, (d, di) in enumerate(phases):
                acts = apool.tile([128, NGT, NCOL], bf, tag="acts")
                for p in PEMIT[:3]:                       # g, i, f
                    emit_pair(head, p, acts)
                emit_chain_pre(d, di, acts)
                nxt = (emit_head(*phases[idx + 1])
                       if idx + 1 < len(phases) else None)
                emit_pair(head, PEMIT[3], acts)           # o
                emit_chain(d, di, acts)
                head = nxt

    nc.finalize()
    return nc


def _get_compiled(nsteps=WD):
    if nsteps not in _COMPILED:
        _COMPILED[nsteps] = _build(nsteps)
    return _COMPILED[nsteps]


# ------------------------------------------------------------------- driver

def kernel(x, w_i2s, b_i2s, w_ih, b_ih, b_hh, k0, k1, b_s2s):
    from concourse.bass_utils import run_bass_kernel_spmd

    in_maps = _prep_inputs(np.asarray(x, np.float32), np.asarray(w_i2s),
                           np.asarray(b_i2s), np.asarray(w_ih),
                           np.asarray(b_ih), np.asarray(b_hh),
                           np.asarray(k0), np.asarray(k1), np.asarray(b_s2s))
    nc = _get_compiled()
    res = run_bass_kernel_spmd(nc, in_maps, list(range(NCORES)))
    return _assemble([res.results[c]["hs"] for c in range(NCORES)])


def kernel_numpy(x, w_i2s, b_i2s, w_ih, b_ih, b_hh, k0, k1, b_s2s):
    """Host-only replica of the device program (debug path)."""
    in_maps = _prep_inputs(np.asarray(x, np.float32), np.asarray(w_i2s),
                           np.asarray(b_i2s), np.asarray(w_ih),
                           np.asarray(b_ih), np.asarray(b_hh),
                           np.asarray(k0), np.asarray(k1), np.asarray(b_s2s))
    return _assemble([_core_sim(im) for im in in_maps])
